# revision 19
# baseline (speedup 1.0000x reference)
"""Trainium2 Bass kernel for nn_AtomEncoder (gnn_message_passing).

Strategy (8 NeuronCores, data-parallel over batch, 4 molecules/core):
  - embedding lookups as a multi-hot matmul: emb^T[d,l] = tabs^T @ MH with
    MH a host-built 0/1 matrix (3 ones per token column); pe added on the
    PSUM->SBUF move.  No gathers, no transposes, PE starts immediately.
  - MLP (two residual 1x1-conv blocks): bf16 PE matmuls in [d, l]
    orientation, weight-stationary inner ordering across molecules.
  - head computed transposed: msgT[l,d] = x2^T @ W5^T (x2 stationary).
  - bond aggregation transposed: aggT[d,l] = msgT^T @ A^T (A host-built
    dense one-hot adjacency, diagonal zeroed); out^T = aggT + x via DVE.
  - output stored [d, l] bf16 in 4 contiguous blocks, unshuffled on host.
"""

import os
import sys
import types

sys.path.insert(0, "/opt/trn_rl_repo")

import numpy as np
import ml_dtypes

BF16 = ml_dtypes.bfloat16
FP8 = ml_dtypes.float8_e4m3

B, L, D, K, NCORES = 32, 512, 256, 6, 8
BPC = B // NCORES          # batch elements per core
NTOK = 128                 # element vocab
# concatenated table offsets: [elem(128), aroma x charge(26), seg x react(60)]
OFF_AC, OFF_SR = 128, 154
VCAT = 256                 # padded concat table rows (2 partition chunks)

LAST_RESULTS = None        # BassKernelResults of the most recent run (for test.py)


def _install_ntff_hook():
    """The agent image lacks antenv.axon_hooks; synthesize it so
    run_bass_kernel_spmd(trace=True) can profile via NTFF."""
    try:
        from antenv.axon_hooks import get_axon_ntff_profile_hook  # noqa: F401
        return
    except ImportError:
        pass
    try:
        import antenv
    except ImportError:
        return
    m = types.ModuleType("antenv.axon_hooks")
    m._hook = None
    m.set_axon_ntff_profile_hook = lambda h: setattr(m, "_hook", h)
    m.get_axon_ntff_profile_hook = lambda: m._hook
    sys.modules["antenv.axon_hooks"] = m
    antenv.axon_hooks = m
    try:
        if "/root/.axon_site" not in sys.path:
            sys.path.append("/root/.axon_site")
        from trn_agent_boot.trn_boot import _ntff_profile_via_ctypes
        m._hook = _ntff_profile_via_ctypes("/opt/axon/libaxon_pjrt.so")
    except Exception:
        pass


_install_ntff_hook()

import concourse.bacc as bacc           # noqa: E402
import concourse.mybir as mybir         # noqa: E402
import concourse.bass_utils as bass_utils  # noqa: E402
from concourse.tile import TileContext  # noqa: E402

# zero-egress container: skip artifact upload in the trace path
bass_utils.upload_artifacts = lambda tmpdir: f"local:{tmpdir}"

F32 = mybir.dt.float32
BF = mybir.dt.bfloat16
F8 = mybir.dt.float8e4
AF = mybir.ActivationFunctionType
ALU = mybir.AluOpType

_prog_cache = {}


def _build_program(bias_flags):
    """bias_flags: (b1,b2,b3,b4,b5) bools — emit bias matmuls only if nonzero."""
    key = tuple(bias_flags)
    if key in _prog_cache:
        return _prog_cache[key]

    nc = bacc.Bacc("TRN2", target_bir_lowering=False, debug=False,
                   num_devices=NCORES)

    tabsd = nc.dram_tensor("tabs", [128, 2, D], F8, kind="ExternalInput")
    mhd = nc.dram_tensor("mh", [BPC, 128, 2, L], F8, kind="ExternalInput")
    pedld = nc.dram_tensor("pedl", [128, 2, L], BF, kind="ExternalInput")
    w1t = nc.dram_tensor("w1t", [128, 2, 4 * D], F8, kind="ExternalInput")
    w2t = nc.dram_tensor("w2t", [128, 8, D], BF, kind="ExternalInput")
    w3t = nc.dram_tensor("w3t", [128, 2, 4 * D], BF, kind="ExternalInput")
    w4t = nc.dram_tensor("w4t", [128, 8, D], BF, kind="ExternalInput")
    w5t = nc.dram_tensor("w5t", [128, 2, D], BF, kind="ExternalInput")
    atd = nc.dram_tensor("at", [BPC, 128, 4, L], BF, kind="ExternalInput")
    biasd = []
    bdims = [4 * D, D, 4 * D, D, D]
    for i, flag in enumerate(bias_flags):
        biasd.append(
            nc.dram_tensor(f"bias{i + 1}", [1, bdims[i]], BF, kind="ExternalInput")
            if flag else None)
    outp = nc.dram_tensor("out", [BPC, 2, 128, L], BF, kind="ExternalOutput")

    with TileContext(nc) as tc:
        with (
            tc.tile_pool(name="const", bufs=1) as cpool,
            tc.tile_pool(name="apool", bufs=4) as apool,
            tc.tile_pool(name="xpool", bufs=12) as xpool,
            tc.tile_pool(name="x8pool", bufs=4) as x8pool,
            tc.tile_pool(name="ypool", bufs=5) as ypool,
            tc.tile_pool(name="mpool", bufs=4) as mpool,
            tc.tile_pool(name="opool", bufs=4) as opool,
            tc.tile_pool(name="psum", bufs=6, space="PSUM") as ppool,
            tc.tile_pool(name="psum5", bufs=2, space="PSUM") as p5pool,
        ):
            # ---- loads in need-order on the two HW-DGE queues ----
            # Each queue is FIFO (priority = program order) and the two
            # queues split HBM bandwidth, so balance bytes and interleave
            # by first-use time.  Software DGE (gpsimd) unused — it
            # contends for HBM and adds a long queue-drain tail.
            # sync:   tabs, mh0, mh2, mh3, at0-3      (~2.7 MB)
            # scalar: mh1, pedl, w1..w5, biases       (~2.7 MB)
            tabs_sb = cpool.tile([128, 2, D], F8)
            nc.sync.dma_start(out=tabs_sb[:], in_=tabsd[:])
            mh_sb = [cpool.tile([128, 2, L], F8, tag=f"mh{b}", name=f"mh{b}")
                     for b in range(BPC)]
            nc.sync.dma_start(out=mh_sb[0][:], in_=mhd[0])
            nc.scalar.dma_start(out=mh_sb[1][:], in_=mhd[1])
            pedl_sb = cpool.tile([128, 2, L], BF)
            nc.scalar.dma_start(out=pedl_sb[:], in_=pedld[:])
            nc.sync.dma_start(out=mh_sb[2][:], in_=mhd[2])
            nc.sync.dma_start(out=mh_sb[3][:], in_=mhd[3])

            w1s = cpool.tile([128, 2, 4 * D], F8)
            nc.scalar.dma_start(out=w1s[:], in_=w1t[:])
            w2s = cpool.tile([128, 8, D], BF)
            nc.scalar.dma_start(out=w2s[:], in_=w2t[:])
            w3s = cpool.tile([128, 2, 4 * D], BF)
            nc.scalar.dma_start(out=w3s[:], in_=w3t[:])
            w4s = cpool.tile([128, 8, D], BF)
            nc.scalar.dma_start(out=w4s[:], in_=w4t[:])
            w5s = cpool.tile([128, 2, D], BF)
            nc.scalar.dma_start(out=w5s[:], in_=w5t[:])

            at_sb = []
            for b in range(BPC):
                t = apool.tile([128, 4, L], BF, tag="at", name=f"at{b}")
                nc.sync.dma_start(out=t[:], in_=atd[b])
                at_sb.append(t)

            bias_sb = []
            for i, dram in enumerate(biasd):
                if dram is None:
                    bias_sb.append(None)
                else:
                    t = cpool.tile([1, bdims[i]], BF, tag=f"bias{i}")
                    nc.scalar.dma_start(out=t[:], in_=dram[:])
                    bias_sb.append(t)
            any_bias = any(bias_flags)
            if any_bias:
                ones = cpool.tile([1, L], BF)
                nc.vector.memset(ones[:], 1.0)

            # ---- PE p-state warmup: dead matmuls while first DMAs land ----
            wu = cpool.tile([128, L], BF)
            nc.vector.memset(wu[:], 0.0)
            wups = ppool.tile([128, L], F32, tag="mm")
            for _ in range(12):
                nc.tensor.matmul(out=wups[:, 0:128], lhsT=wu[:, 0:128],
                                 rhs=wu[:, 0:128], start=True, stop=True)

            # ---- emb^T = tabs^T @ MH (+pe on the PSUM->SBUF move) ----
            # x[b] bf16 [128(d), 2(dc), 512(l)] — the MLP input AND the
            # emb term of the output (kept resident until the end).
            xs = []
            x8s = []
            for b in range(BPC):
                x = xpool.tile([128, 2, L], BF, tag="x")
                x8 = x8pool.tile([128, 2, L], F8, tag="x8")
                for dc in range(2):
                    ps = ppool.tile([128, L], F32, tag="mm")
                    nc.tensor.matmul(
                        out=ps[:],
                        lhsT=tabs_sb[:, :, dc * 128:(dc + 1) * 128],
                        rhs=mh_sb[b][:],
                        perf_mode=mybir.MatmulPerfMode.DoubleRow,
                        start=True, stop=True)
                    nc.vector.tensor_tensor(
                        out=x[:, dc, :], in0=ps[:], in1=pedl_sb[:, dc, :],
                        op=ALU.add)
                # fp8 copy of x for the DoubleRow L1 input
                nc.scalar.activation(out=x8[:], in_=x[:], func=AF.Copy)
                xs.append(x)
                x8s.append(x8)

            def relu_out(dst, ps, i):
                # split relus roughly evenly across DVE and ACT
                if i % 16 < 7:
                    nc.vector.tensor_scalar(
                        out=dst, in0=ps[:], scalar1=0.0, scalar2=None,
                        op0=ALU.max)
                else:
                    nc.scalar.activation(out=dst, in_=ps[:], func=AF.Relu)

            # expand layer, one molecule (fp8 DoubleRow): y = relu(w x + b)
            def expand8_mol(b, xin, wsb, bsb):
                y = ypool.tile([128, 8, L], BF, tag="y", name=f"y8{b}")
                for m in range(8):
                    ps = ppool.tile([128, L], F32, tag="mm")
                    nc.tensor.matmul(
                        out=ps[:],
                        lhsT=wsb[:, :, m * 128:(m + 1) * 128],
                        rhs=xin[:],
                        perf_mode=mybir.MatmulPerfMode.DoubleRow,
                        start=True, stop=(bsb is None))
                    if bsb is not None:
                        nc.tensor.matmul(
                            out=ps[:],
                            lhsT=bsb[:1, m * 128:(m + 1) * 128],
                            rhs=ones[:1, :],
                            start=False, stop=True)
                    relu_out(y[:, m, :], ps, b * 8 + m)
                return y

            # expand layer, one molecule (bf16): y = relu(w x + b)
            def expand_mol(b, xin, wsb, bsb):
                y = ypool.tile([128, 8, L], BF, tag="y", name=f"y{b}")
                for m in range(8):
                    ps = ppool.tile([128, L], F32, tag="mm")
                    for kc in range(2):
                        nc.tensor.matmul(
                            out=ps[:],
                            lhsT=wsb[:, kc, m * 128:(m + 1) * 128],
                            rhs=xin[:, kc, :],
                            start=(kc == 0),
                            stop=(kc == 1 and bsb is None))
                    if bsb is not None:
                        nc.tensor.matmul(
                            out=ps[:],
                            lhsT=bsb[:1, m * 128:(m + 1) * 128],
                            rhs=ones[:1, :],
                            start=False, stop=True)
                    relu_out(y[:, m, :], ps, b * 8 + m)
                return y

            # contract layer, one molecule: xnew = xres + w y + b
            def contract_mol(b, y, wsb, bsb, xres):
                xn = xpool.tile([128, 2, L], BF, tag="x", name=f"xn{b}")
                for m in range(2):
                    ps = ppool.tile([128, L], F32, tag="mm")
                    for kc in range(8):
                        nc.tensor.matmul(
                            out=ps[:],
                            lhsT=wsb[:, kc, m * 128:(m + 1) * 128],
                            rhs=y[:, kc, :],
                            start=(kc == 0),
                            stop=(kc == 7 and bsb is None))
                    if bsb is not None:
                        nc.tensor.matmul(
                            out=ps[:],
                            lhsT=bsb[:1, m * 128:(m + 1) * 128],
                            rhs=ones[:1, :],
                            start=False, stop=True)
                    nc.vector.tensor_tensor(
                        out=xn[:, m, :], in0=ps[:],
                        in1=xres[:, m, :], op=ALU.add)
                return xn

            # block 1, software-pipelined per molecule: the contract
            # layer's PE work hides the expand layer's PSUM drain.
            y1s = [None] * BPC
            x1s = [None] * BPC
            y1s[0] = expand8_mol(0, x8s[0], w1s, bias_sb[0])
            y1s[1] = expand8_mol(1, x8s[1], w1s, bias_sb[0])
            x1s[0] = contract_mol(0, y1s[0], w2s, bias_sb[1], xs[0])
            y1s[2] = expand8_mol(2, x8s[2], w1s, bias_sb[0])
            x1s[1] = contract_mol(1, y1s[1], w2s, bias_sb[1], xs[1])
            y1s[3] = expand8_mol(3, x8s[3], w1s, bias_sb[0])
            x1s[2] = contract_mol(2, y1s[2], w2s, bias_sb[1], xs[2])
            x1s[3] = contract_mol(3, y1s[3], w2s, bias_sb[1], xs[3])

            # block 2
            y3s = [None] * BPC
            x2s = [None] * BPC
            y3s[0] = expand_mol(0, x1s[0], w3s, bias_sb[2])
            y3s[1] = expand_mol(1, x1s[1], w3s, bias_sb[2])
            x2s[0] = contract_mol(0, y3s[0], w4s, bias_sb[3], x1s[0])
            y3s[2] = expand_mol(2, x1s[2], w3s, bias_sb[2])
            x2s[1] = contract_mol(1, y3s[1], w4s, bias_sb[3], x1s[1])
            y3s[3] = expand_mol(3, x1s[3], w3s, bias_sb[2])
            x2s[2] = contract_mol(2, y3s[2], w4s, bias_sb[3], x1s[2])
            x2s[3] = contract_mol(3, y3s[3], w4s, bias_sb[3], x1s[3])

            # head (transposed): msgT[l, d] = x2^T @ W5^T; then
            # aggT[d, l] = msgT^T @ A^T; out^T = aggT + x.
            def head(b):
                msgT = mpool.tile([128, 4, D], BF, tag="msgT")
                for jc in range(4):
                    ps = p5pool.tile([128, D], F32, tag="p5")
                    for dc in range(2):
                        nc.tensor.matmul(
                            out=ps[:],
                            lhsT=x2s[b][:, dc, jc * 128:(jc + 1) * 128],
                            rhs=w5s[:, dc, :],
                            start=(dc == 0),
                            stop=(dc == 1 and bias_sb[4] is None))
                    if bias_sb[4] is not None:
                        nc.tensor.matmul(
                            out=ps[:],
                            lhsT=ones[:1, jc * 128:(jc + 1) * 128],
                            rhs=bias_sb[4][:1, :],
                            start=False, stop=True)
                    if jc % 2:
                        nc.vector.tensor_copy(out=msgT[:, jc, :], in_=ps[:])
                    else:
                        nc.scalar.activation(out=msgT[:, jc, :], in_=ps[:],
                                             func=AF.Copy)
                return msgT

            def agg(b, msgT):
                ot = opool.tile([128, 2, L], BF, tag="ot")
                for dc in range(2):
                    ps = ppool.tile([128, L], F32, tag="mm")
                    for jc in range(4):
                        nc.tensor.matmul(
                            out=ps[:],
                            lhsT=msgT[:, jc, dc * 128:(dc + 1) * 128],
                            rhs=at_sb[b][:, jc, :],
                            start=(jc == 0), stop=(jc == 3))
                    nc.vector.tensor_tensor(
                        out=ot[:, dc, :], in0=ps[:], in1=xs[b][:, dc, :],
                        op=ALU.add)
                    # store each half as soon as it's ready (shorter tail)
                    nc.sync.dma_start(out=outp[b, dc], in_=ot[:, dc, :])

            # software-pipeline the head: msgT[b+1] copies overlap aggT[b]
            msgTs = [head(0), head(1)]
            agg(0, msgTs[0])
            msgTs.append(head(2))
            agg(1, msgTs[1])
            msgTs.append(head(3))
            agg(2, msgTs[2])
            agg(3, msgTs[3])

    nc.compile()
    _prog_cache[key] = nc
    return nc


def _host_prep(inp):
    """Build per-core in_maps."""
    element = np.asarray(inp["element"]).astype(np.int64)
    bond = np.asarray(inp["bond"]).astype(np.int64)
    aroma = np.asarray(inp["aroma"]).astype(np.int64)
    charge = np.asarray(inp["charge"]).astype(np.int64)
    segment = np.asarray(inp["segment"]).astype(np.int64)
    react = np.asarray(inp["reactant_mask"]).astype(np.int64)

    tab = np.zeros((VCAT, D), np.float32)
    tab[0:128] = np.asarray(inp["elem_emb"])
    ar = np.asarray(inp["aroma_emb"], dtype=np.float32)
    ch = np.asarray(inp["charge_emb"], dtype=np.float32)
    sg = np.asarray(inp["seg_emb"], dtype=np.float32)
    rc = np.asarray(inp["react_emb"], dtype=np.float32)
    tab[OFF_AC:OFF_AC + 26] = (ar[:, None, :] + ch[None, :, :]).reshape(26, D)
    tab[OFF_SR:OFF_SR + 60] = (sg[:, None, :] + rc[None, :, :]).reshape(60, D)
    tabs_host = np.ascontiguousarray(
        tab.reshape(2, 128, D).transpose(1, 0, 2)).astype(FP8)

    pe = np.asarray(inp["pe"]).reshape(L, D).astype(np.float32)
    pedl_host = np.ascontiguousarray(
        pe.T.reshape(2, 128, L).transpose(1, 0, 2)).astype(BF16)

    def wprep(w, kchunks):  # w [dout, din] -> [128, kchunks, dout] bf16
        wT = np.asarray(w).T  # [din, dout]
        return np.ascontiguousarray(
            wT.reshape(kchunks, 128, wT.shape[1]).transpose(1, 0, 2)).astype(BF16)

    w1t = wprep(inp["w1"], 2).astype(FP8)
    w2t = wprep(inp["w2"], 8)
    w3t = wprep(inp["w3"], 2)
    w4t = wprep(inp["w4"], 8)
    w5t = wprep(inp["w5"], 2)

    biases = [np.asarray(inp[f"b{i}"]).astype(np.float32) for i in range(1, 6)]
    bias_flags = tuple(bool(np.any(b != 0.0)) for b in biases)
    bias_rows = [b.reshape(1, -1).astype(BF16) for b in biases]

    # multi-hot [B, VCAT, L]: 3 disjoint one rows per token column
    rows = np.stack([
        element,
        OFF_AC + aroma * 13 + (charge + 6),
        OFF_SR + segment * 2 + react,
    ])  # [3, B, L]
    mh = np.zeros((B, VCAT, L), np.float32)
    bidx = np.arange(B)[None, :, None]
    lidx = np.arange(L)[None, None, :]
    mh[bidx, rows, lidx] = 1.0
    mh_host = np.ascontiguousarray(
        mh.reshape(B, 2, 128, L).transpose(0, 2, 1, 3)).astype(FP8)

    # adjacency A^T per molecule
    lr = np.arange(L)
    lrep = np.repeat(lr, K)
    in_maps = []
    for c in range(NCORES):
        at = np.empty((BPC, 128, 4, L), np.float32)
        for bl, bg in enumerate(range(c * BPC, (c + 1) * BPC)):
            A = np.zeros((L, L), np.float32)
            np.add.at(A, (lrep, bond[bg].ravel()), 1.0)
            A[lr, lr] = 0.0
            at[bl] = A.T.reshape(4, 128, L).transpose(1, 0, 2)
        m = {
            "tabs": tabs_host,
            "mh": mh_host[c * BPC:(c + 1) * BPC],
            "pedl": pedl_host,
            "w1t": w1t, "w2t": w2t, "w3t": w3t, "w4t": w4t, "w5t": w5t,
            "at": at.astype(BF16),
        }
        for i, flag in enumerate(bias_flags):
            if flag:
                m[f"bias{i + 1}"] = bias_rows[i]
        in_maps.append(m)
    return in_maps, bias_flags


def kernel(**inputs):
    global LAST_RESULTS
    from concourse.bass_utils import run_bass_kernel_spmd
    in_maps, bias_flags = _host_prep(inputs)
    nc = _build_program(bias_flags)
    trace = bool(int(os.environ.get("ATOM_TRACE", "0")))
    res = run_bass_kernel_spmd(nc, in_maps, list(range(NCORES)), trace=trace)
    LAST_RESULTS = res
    out = np.empty((L, B, D), np.float32)
    for c in range(NCORES):
        o = np.asarray(res.results[c]["out"]).astype(np.float32)
        # o[b, dc, p, l] -> out[l, b, dc*128+p]
        out[:, c * BPC:(c + 1) * BPC, :] = o.transpose(3, 0, 1, 2).reshape(
            L, BPC, D)
    return out
